# revision 1
# baseline (speedup 1.0000x reference)
"""Trainium2 Bass kernel for weighted-CE + structural-penalty loss.

Full inputs -> data-parallel shard over batch across 8 NeuronCores ->
per-core Bass kernel computes small partial sums -> host combines the
(tiny) partials in float64.

CE:  -mean(w[t] * log_softmax(logits)[t]) = (1/N) sum_c w_c (W_c - S_c),
  W_c = sum_pos 1[t==c]*lse,  S_c = sum_pos 1[t==c]*x_c.
  An interleaved one-hot M[p, j*8+c] = (t==c) (fp16, one 2x-mode
  tensor_tensor per chunk from a GPSIMD-replicated int16 target) feeds:
   - lse side: matmul(lhsT=lse-block, rhs=M window) accumulating a
     shifted diagonal in one PSUM bank, classes separated by col%8;
   - x side:  MX = M * Xh elementwise, then ones-matmuls column-reduce
     into a [1, 512] PSUM (fold j%64, classes by col%8);
   - nnz: ones-matmul over M's class-0 stride-8 columns.
  Host extracts the diagonals/columns and applies weights in float64.

Penalty: per row, pen = pair_sum + P_final - 2*min(0, min_prefix(P)) with
  P = cumsum((s==1)-(s==2)) via the hardware tensor_tensor_scan; pair
  terms are shifted-mask products reduced by ones-matmuls.  Rows are
  split into two 2048-halves on partitions r | 64+r (first half has a
  3-column real halo, second a zero halo); host chains the halves and
  adds the one genuinely-clamped boundary term.
"""

import numpy as np

import concourse.bass as bass
import concourse.mybir as mybir
import concourse.tile as tile
from concourse import bacc
from concourse.bass_utils import run_bass_kernel_spmd

B, S, C = 512, 4096, 8
PENALTY_WEIGHT = 0.1
NCORES = 8
RB = B // NCORES          # rows (batch) per core
N = RB * S                # positions per core
P = 128                   # SBUF partitions
NP = N // P               # positions per partition
NCH = 4                   # CE processed in NCH free-dim chunks
PCH = NP // NCH           # positions per partition per chunk (512)
NW = PCH // 64            # 64-position rhs windows per chunk (8)
SH = S // 2               # penalty half-row length
HALO = 3

F32 = mybir.dt.float32
F16 = mybir.dt.float16
I32 = mybir.dt.int32
I16 = mybir.dt.int16
OP = mybir.AluOpType
AF = mybir.ActivationFunctionType


def _patch_act_tables():
    """Prefer the single table set containing Exp+Ln+Copy so the kernel
    pays one ACT_TABLE_LOAD instead of alternating per chunk.  Set ids
    are positional, so blank out other sets rather than reordering."""
    import concourse.hw_specs as hw_specs
    if getattr(hw_specs, "_loss_kernel_tables_patched", False):
        return
    orig = hw_specs.get_activation_tables

    def patched(arch):
        t = orig(arch)
        pref = "natural_log_exp_and_others"
        if pref not in t:
            return t
        return {k: (v if k == pref else set()) for k, v in t.items()}

    hw_specs.get_activation_tables = patched
    bacc.get_activation_tables = patched
    hw_specs._loss_kernel_tables_patched = True


USE_TABLE_PATCH = True


def build_program(compile=True):
    if USE_TABLE_PATCH:
        _patch_act_tables()
    nc = bacc.Bacc("TRN2", target_bir_lowering=False, debug=False)

    logits_d = nc.dram_tensor("logits", [P, NP * C], F32, kind="ExternalInput").ap()
    targets_d = nc.dram_tensor("targets", [P, NP], I32, kind="ExternalInput").ap()
    structs_d = nc.dram_tensor("structs", [RB, S], I32, kind="ExternalInput").ap()

    dlse_d = nc.dram_tensor("diag_lse", [64, 512], F32, kind="ExternalOutput").ap()
    dx_d = nc.dram_tensor("diag_x", [P, 8, P], F32, kind="ExternalOutput").ap()
    vec_d = nc.dram_tensor("vec_acc", [1, 4, 512], F32, kind="ExternalOutput").ap()
    pen_scan_d = nc.dram_tensor("pen_scan", [P, 2], F32, kind="ExternalOutput").ap()

    SW = SH + HALO

    with tile.TileContext(nc) as tc:
        with (
            tc.tile_pool(name="big", bufs=2) as big,
            tc.tile_pool(name="ebuf", bufs=1) as ebuf,
            tc.tile_pool(name="mid", bufs=1) as mid,
            tc.tile_pool(name="lsep", bufs=2) as lsep,
            tc.tile_pool(name="mip", bufs=2) as mip,
            tc.tile_pool(name="pen", bufs=1) as pen,
            tc.tile_pool(name="acc", bufs=1) as acc,
            tc.tile_pool(name="psum", bufs=1, space="PSUM") as psum,
        ):
            # psum accumulators
            ps_lse = psum.tile([64, 512], F32, name="ps_lse")
            ps_x = [psum.tile([P, 4, P], F32, name=f"ps_x{q}") for q in range(2)]
            ps_vec = [psum.tile([1, 512], F32, name=f"ps_vec{i}") for i in range(4)]
            # ps_vec: 0=cnt0, 1=pair2, 2=pair3, 3=pair4
            started = set()

            def acc_mm(key, out, lhsT, rhs, last):
                st = key not in started
                started.add(key)
                nc.tensor.matmul(out, lhsT=lhsT, rhs=rhs, start=st, stop=last)

            ones_t = acc.tile([P, 1], F16)
            nc.vector.memset(ones_t, 1.0)

            t_sb = pen.tile([P, NP], I32)
            nc.sync.dma_start(out=t_sb, in_=targets_d)

            # ---------------- CE chunks ----------------
            for k in range(NCH):
                fl = k * PCH * C
                x_t = big.tile([P, PCH * C], F32, tag="x")
                nc.sync.dma_start(out=x_t, in_=logits_d[:, fl : fl + PCH * C])

                # class-blocked masks first: DVE fills the exp wait
                m2 = mip.tile([P, C, PCH], F16, tag="m2")
                tk = t_sb[:, k * PCH : (k + 1) * PCH]
                for c in range(C):
                    nc.vector.tensor_scalar(out=m2[:, c, :], in0=tk,
                                            scalar1=float(c), scalar2=None,
                                            op0=OP.is_equal)

                e_x = ebuf.tile([P, PCH * C], F16, tag="e")
                nc.scalar.activation(e_x, x_t, AF.Exp)
                e3 = e_x.rearrange("p (n c) -> p n c", c=C)
                s4 = mid.tile([P, PCH, 4], F16, tag="s4")
                nc.vector.tensor_add(s4, e3[:, :, 0:4], e3[:, :, 4:8])
                s2 = mid.tile([P, PCH, 2], F16, tag="s2")
                nc.vector.tensor_add(s2, s4[:, :, 0:2], s4[:, :, 2:4])
                se = mid.tile([P, PCH], F16, tag="se")
                se3 = se.rearrange("p (n o) -> p n o", o=1)
                nc.vector.tensor_add(se3, s2[:, :, 0:1], s2[:, :, 1:2])
                lse = lsep.tile([P, PCH], F16, tag="lse")
                nc.scalar.activation(lse, se, AF.Ln)
                xh = ebuf.tile([P, PCH * C], F16, tag="xh")
                nc.scalar.activation(xh, x_t, AF.Copy)  # fp32 -> fp16 cast
                xh3 = xh.rearrange("p (n c) -> p n c", c=C)

                last = k == NCH - 1
                # lse side: 64-position windows; rhs gathers all 8 class
                # slices for the window -> permuted diagonal, all rows useful
                for w in range(NW):
                    rhs = bass.AP(
                        tensor=m2.tensor, offset=m2.offset + w * 64,
                        ap=[m2.ap[0], [PCH, C], [1, 64]])
                    acc_mm(("lse",), ps_lse,
                           lhsT=lse[:, w * 64 : (w + 1) * 64], rhs=rhs,
                           last=last and w == NW - 1)

                # x side: per-class diagonal psums (4 classes per bank)
                for c in range(C):
                    q, sl = divmod(c, 4)
                    for b in range(PCH // P):
                        bs = slice(b * P, (b + 1) * P)
                        acc_mm(("x", q), ps_x[q][:, sl, :],
                               lhsT=m2[:, c, bs], rhs=xh3[:, bs, c],
                               last=(last and c in (3, 7) and b == PCH // P - 1))

                # count of t==0: ones-matmul over the class-0 mask block
                acc_mm(("cnt",), ps_vec[0], lhsT=ones_t, rhs=m2[:, 0, :],
                       last=last)

            # -------- penalty: row halves on partitions (r | 64+r) --------
            s_t = pen.tile([P, SW], I32)
            nc.sync.dma_start(out=s_t[0:RB, :], in_=structs_d[:, 0:SW])
            nc.sync.dma_start(out=s_t[RB:P, 0:SH], in_=structs_d[:, SH:S])
            nc.vector.memset(s_t[RB:P, SH:SW], 0)

            lp_t = pen.tile([P, SW], F16)
            r_t = pen.tile([P, SW], F16)
            e_t = pen.tile([P, SW], F16)
            nc.vector.tensor_scalar(out=lp_t, in0=s_t, scalar1=1.0, scalar2=None,
                                    op0=OP.is_equal)
            nc.vector.tensor_scalar(out=r_t, in0=s_t, scalar1=2.0, scalar2=None,
                                    op0=OP.is_equal)
            nc.vector.tensor_scalar(out=e_t, in0=s_t, scalar1=3.0, scalar2=None,
                                    op0=OP.is_equal)

            p_t = pen.tile([P, SH], F32)
            nc.vector.tensor_tensor_scan(out=p_t, data0=lp_t[:, 0:SH],
                                         data1=r_t[:, 0:SH], initial=0.0,
                                         op0=OP.add, op1=OP.subtract)
            scan_out = acc.tile([P, 2], F32)
            nc.vector.tensor_copy(out=scan_out[:, 0:1], in_=p_t[:, SH - 1 : SH])
            nc.vector.tensor_reduce(out=scan_out[:, 1:2], in_=p_t,
                                    axis=mybir.AxisListType.X, op=OP.min)
            nc.sync.dma_start(out=pen_scan_d, in_=scan_out)

            # er[j]=e[j]*r[j+1]; eer[j]=e[j]*er[j+1]; pair products with lp
            er_t = pen.tile([P, SW], F16)
            eer_t = pen.tile([P, SW], F16)
            nc.vector.tensor_mul(er_t[:, 0 : SW - 1], e_t[:, 0 : SW - 1], r_t[:, 1:SW])
            nc.vector.tensor_mul(eer_t[:, 0 : SW - 2], e_t[:, 0 : SW - 2],
                                 er_t[:, 1 : SW - 1])
            pr2 = pen.tile([P, SH], F16)
            pr3 = pen.tile([P, SH], F16)
            pr4 = pen.tile([P, SH], F16)
            nc.vector.tensor_mul(pr2, lp_t[:, 0:SH], r_t[:, 1 : SH + 1])
            nc.vector.tensor_mul(pr3, lp_t[:, 0:SH], er_t[:, 1 : SH + 1])
            nc.vector.tensor_mul(pr4, lp_t[:, 0:SH], eer_t[:, 1 : SH + 1])
            for i, pr in ((1, pr2), (2, pr3), (3, pr4)):
                for w in range(SH // 512):
                    acc_mm((f"p{i}",), ps_vec[i], lhsT=ones_t,
                           rhs=pr[:, w * 512 : (w + 1) * 512],
                           last=w == SH // 512 - 1)

            # -------- dump psums --------
            dl_sb = acc.tile([64, 512], F32)
            nc.scalar.activation(dl_sb, ps_lse, AF.Copy)
            nc.sync.dma_start(out=dlse_d, in_=dl_sb)
            dx_sb = acc.tile([P, 8, P], F32)
            for q in range(2):
                nc.scalar.activation(dx_sb[:, q * 4 : (q + 1) * 4, :],
                                     ps_x[q][:, :, :], AF.Copy)
            nc.sync.dma_start(out=dx_d, in_=dx_sb)
            vec_sb = acc.tile([1, 4, 512], F32)
            for i in range(4):
                nc.scalar.activation(vec_sb[:, i, :], ps_vec[i], AF.Copy)
            nc.sync.dma_start(out=vec_d, in_=vec_sb)

    if compile:
        nc.compile()
    return nc


_program = None


def _get_program():
    global _program
    if _program is None:
        _program = build_program()
    return _program


def _pair_boundary(s):
    """The only clamped boundary pair term not covered on device:
    4 * [s[S-3]==1][s[S-2]==3][s[S-1]==2] per row."""
    m = (s[:, -3] == 1) & (s[:, -2] == 3) & (s[:, -1] == 2)
    return 4.0 * float(m.sum())


def combine_partials(results, s_full, ce_weights):
    """Host-side (float64) combination of per-core device partials."""
    w = np.asarray(ce_weights, np.float64)
    Wc = np.zeros(C, np.float64)
    Sc = np.zeros(C, np.float64)
    z0 = 0.0
    pen = 0.0
    r_idx = np.arange(64)
    p_idx = np.arange(P)
    for r in results:
        dl = r["diag_lse"].astype(np.float64)   # [64, 512]
        for c in range(C):
            Wc[c] += dl[r_idx, c * 64 + r_idx].sum()
        dx = r["diag_x"].astype(np.float64)     # [128, 8, 128]
        Sc += dx[p_idx, :, p_idx].sum(0)
        va = r["vec_acc"].astype(np.float64).reshape(4, 512)
        z0 += va[0].sum()
        pen += 2.0 * va[1].sum() + 3.0 * va[2].sum() + 4.0 * va[3].sum()
        sc = r["pen_scan"].astype(np.float64)
        pfa, mpa = sc[0:RB, 0], sc[0:RB, 1]
        pfb, mpb = sc[RB:P, 0], sc[RB:P, 1]
        pf = pfa + pfb
        mp = np.minimum(mpa, pfa + mpb)
        pen += (pf - 2.0 * np.minimum(0.0, mp)).sum()
    pen += _pair_boundary(s_full)
    ce_loss = float((w * (Wc - Sc)).sum()) / (B * S)
    nnz = B * S - z0
    penalty = pen / nnz
    return np.float32(ce_loss + PENALTY_WEIGHT * penalty)


def make_in_maps(logits, targets, predicted_structures):
    lg = np.ascontiguousarray(logits, dtype=np.float32)
    t = np.ascontiguousarray(targets, dtype=np.int32)
    s = np.ascontiguousarray(predicted_structures.reshape(B, S), dtype=np.int32)
    in_maps = []
    for core in range(NCORES):
        rows = slice(core * RB, (core + 1) * RB)
        in_maps.append({
            "logits": lg[rows].reshape(P, NP * C),
            "targets": t[rows].reshape(P, NP),
            "structs": s[rows],
        })
    return in_maps, s


def kernel(logits, targets, predicted_structures, ce_weights):
    in_maps, s = make_in_maps(logits, targets, predicted_structures)
    nc = _get_program()
    res = run_bass_kernel_spmd(nc, in_maps, core_ids=list(range(NCORES)))
    return combine_partials(res.results, s, ce_weights)



# revision 20
# speedup vs baseline: 1.4300x; 1.4300x over previous
"""Trainium2 Bass kernel for weighted-CE + structural-penalty loss.

Full inputs -> data-parallel shard over batch across 8 NeuronCores ->
per-core Bass kernel computes small partial sums -> host combines the
(tiny) partials in float64.

CE: -mean(w[t] * log_softmax(logits)[t]) = (1/N) sum_c w_c (W_c - S_c),
  W_c = sum_pos 1[t==c]*lse,  S_c = sum_pos 1[t==c]*x_c.
  Logits arrive via a SWDGE cast-DMA as fp16 (HBM read stays fp32-sized).
  ScalarE does Exp; DVE does a 3-level pairwise tree for sum-exp;
  ScalarE does Ln. Class one-hot planes m2[c] (fp16, 8 tensor_scalar
  is_equal ops) feed two PE diagonal-window contractions:
   - W side: lhsT = 64-position lse window, rhs = all 8 class planes of
     that window (512 cols) accumulating one [64, 512] PSUM; diagonals
     hold per-class lse sums.
   - S side: lhsT = 128-logit xh window (16 pos x 8 cls, FWL-eligible),
     rhs = class planes of the window (128 cols) accumulating one
     [128, 128] PSUM; "diagonals" hold per-class x sums.
  Host extracts diagonals and applies ce_weights in float64.
  Count of t==0 comes from a tensor_scalar is_equal with accum_out.

Penalty: per row, pen = pair_sum + P_final - 2*min(0, min_prefix(P)) with
  P = cumsum((s==1)-(s==2)) via tensor_tensor_scan (on GpSimd); min via a
  tensor_tensor_reduce with min-accumulate (init 0). Pair terms use a
  nibble encoding v_j = s_j + 4 s_{j+1} + 16 s_{j+2} + 64 s_{j+3} built
  with two fused scalar_tensor_tensor ops; each pair sum is ONE fused
  tensor_scalar ((v&15)==9 etc.) with accum_out. Rows are split into two
  2048-halves on partitions r | 64+r (first half real 3-col halo, second
  zero halo); host chains the halves and adds the one clamped boundary
  term.
"""

import numpy as np

import concourse.bass as bass
import concourse.mybir as mybir
import concourse.tile as tile
from concourse import bacc
from concourse.bass_utils import run_bass_kernel_spmd
from concourse.dve_ops import AFFINE_THEN_ADD

B, S, C = 512, 4096, 8
PENALTY_WEIGHT = 0.1
NCORES = 8
RB = B // NCORES          # rows (batch) per core
N = RB * S                # positions per core
P = 128                   # SBUF partitions
NP = N // P               # positions per partition (2048)
NCH = 4                   # CE processed in NCH chunks
PCH = NP // NCH           # positions per partition per chunk (512)
SH = S // 2               # penalty half-row length (2048)
SW = SH + 4               # struct cols sent per partition (halo + pad)

F32 = mybir.dt.float32
F16 = mybir.dt.float16
U16 = mybir.dt.uint16
OP = mybir.AluOpType
AF = mybir.ActivationFunctionType


def _patch_act_tables():
    """Prefer the single table set containing Exp+Ln+Copy so the kernel
    pays one ACT_TABLE_LOAD instead of alternating per chunk.  Set ids
    are positional, so blank out other sets rather than reordering."""
    import concourse.hw_specs as hw_specs
    if getattr(hw_specs, "_loss_kernel_tables_patched", False):
        return
    orig = hw_specs.get_activation_tables

    def patched(arch):
        t = orig(arch)
        pref = "natural_log_exp_and_others"
        if pref not in t:
            return t
        return {k: (v if k == pref else set()) for k, v in t.items()}

    hw_specs.get_activation_tables = patched
    bacc.get_activation_tables = patched
    hw_specs._loss_kernel_tables_patched = True


def build_program(compile=True):
    _patch_act_tables()
    nc = bacc.Bacc("TRN2", target_bir_lowering=False, debug=False)

    logits_d = nc.dram_tensor("logits", [P, NP * C], F32, kind="ExternalInput").ap()
    targets_d = nc.dram_tensor("targets", [P, NP], U16, kind="ExternalInput").ap()
    structs_d = nc.dram_tensor("structs", [P, SW], U16, kind="ExternalInput").ap()

    wps_d = nc.dram_tensor("w_ps", [64, 512], F32, kind="ExternalOutput").ap()
    sps_d = nc.dram_tensor("s_ps", [P, P], F32, kind="ExternalOutput").ap()
    acc_d = nc.dram_tensor("acc", [P, 8], F32, kind="ExternalOutput").ap()

    with tile.TileContext(nc) as tc:
        with (
            tc.tile_pool(name="xh", bufs=2) as xhp,
            tc.tile_pool(name="e", bufs=2) as ep,
            tc.tile_pool(name="tree", bufs=2) as treep,
            tc.tile_pool(name="lse", bufs=2) as lsep,
            tc.tile_pool(name="m2", bufs=1) as m2p,
            tc.tile_pool(name="pen", bufs=1) as pen,
            tc.tile_pool(name="acc", bufs=1) as accp,
            tc.tile_pool(name="psum", bufs=1, space="PSUM") as psum,
        ):
            w_ps = psum.tile([64, 512], F32, name="w_ps")
            s_ps = psum.tile([P, P], F32, name="s_ps")
            started = set()

            def acc_mm(key, out, lhsT, rhs, last):
                st = key not in started
                started.add(key)
                nc.tensor.matmul(out, lhsT=lhsT, rhs=rhs, start=st, stop=last)

            acc_t = accp.tile([P, 8], F32)
            junk = accp.tile([P, NP], F16)
            ones_t = accp.tile([P, NP], F16)
            nc.vector.memset(ones_t, 1.0)

            t_sb = pen.tile([P, NP], U16)
            nc.sync.dma_start(out=t_sb, in_=targets_d)
            s_sb = pen.tile([P, SW], U16)
            nc.sync.dma_start(out=s_sb, in_=structs_d)

            # one-hot class planes [P, C, NP] fp16
            m2 = m2p.tile([P, C, NP], F16)
            for c in range(C):
                nc.vector.tensor_scalar(out=m2[:, c, :], in0=t_sb,
                                        scalar1=float(c), scalar2=None,
                                        op0=OP.is_equal)
            # count of t==0 (fused accumulate)
            nc.vector.tensor_scalar(out=junk, in0=t_sb, scalar1=0.0,
                                    scalar2=None, op0=OP.is_equal,
                                    op1=OP.add, accum_out=acc_t[:, 0:1])

            # ---------------- CE chunks ----------------
            for k in range(NCH):
                fl = k * PCH * C
                xh = xhp.tile([P, PCH * C], F16, tag="xh")
                # SWDGE cast-DMA: HBM fp32 -> SBUF fp16
                nc.gpsimd.dma_start(out=xh, in_=logits_d[:, fl:fl + PCH * C])

                e_x = ep.tile([P, PCH * C], F16, tag="e")
                nc.scalar.activation(e_x, xh, AF.Exp)
                e3 = e_x.rearrange("p (n c) -> p n c", c=C)
                a_t = treep.tile([P, PCH, 4], F16, tag="a")
                nc.vector.tensor_add(a_t, e3[:, :, 0:4], e3[:, :, 4:8])
                b_t = treep.tile([P, PCH, 2], F16, tag="b")
                nc.vector.tensor_add(b_t, a_t[:, :, 0:2], a_t[:, :, 2:4])
                se = treep.tile([P, PCH], F16, tag="se")
                se3 = se.rearrange("p (n o) -> p n o", o=1)
                nc.vector.tensor_add(se3, b_t[:, :, 0:1], b_t[:, :, 1:2])
                lse = lsep.tile([P, PCH], F16, tag="lse")
                nc.scalar.activation(lse, se, AF.Ln)

                last = k == NCH - 1
                # W side: 64-position windows, rhs = 8 class planes (512 cols)
                for w in range(PCH // 64):
                    j0 = k * PCH + w * 64
                    acc_mm(("w",), w_ps,
                           lhsT=lse[:, w * 64:(w + 1) * 64],
                           rhs=m2[:, :, j0:j0 + 64],
                           last=last and w == PCH // 64 - 1)
                # S side: 16-position windows, lhsT = contiguous 128-col xh
                for w in range(PCH // 16):
                    j0 = k * PCH + w * 16
                    acc_mm(("s",), s_ps,
                           lhsT=xh[:, w * 128:(w + 1) * 128],
                           rhs=m2[:, :, j0:j0 + 16],
                           last=last and w == PCH // 16 - 1)

            # -------- penalty: row halves on partitions (r | 64+r) --------
            lp_t = pen.tile([P, SH], F16)
            rp_t = pen.tile([P, SH], F16)
            nc.vector.tensor_scalar(out=lp_t, in0=s_sb[:, 0:SH], scalar1=1.0,
                                    scalar2=None, op0=OP.is_equal)
            nc.vector.tensor_scalar(out=rp_t, in0=s_sb[:, 0:SH], scalar1=2.0,
                                    scalar2=None, op0=OP.is_equal)
            p_t = pen.tile([P, SH], F32)
            nc.vector.tensor_tensor_scan(out=p_t, data0=lp_t, data1=rp_t,
                                         initial=0.0, op0=OP.add,
                                         op1=OP.subtract)
            # raw min-prefix; host applies min(0, .)
            nc.vector.tensor_reduce(out=acc_t[:, 4:5], in_=p_t,
                                    axis=mybir.AxisListType.X, op=OP.min)
            nc.vector.tensor_copy(out=acc_t[:, 5:6], in_=p_t[:, SH - 1:SH])

            # nibble encoding v_j = s_j + 4 s_{j+1} + 16 s_{j+2} + 64 s_{j+3}
            u_t = pen.tile([P, SH + 2], F16)
            nc.vector._custom_dve(AFFINE_THEN_ADD, out=u_t,
                                  in0=s_sb[:, 1:SH + 3], in1=s_sb[:, 0:SH + 2],
                                  s0=4.0, s1=0.0)
            v_t = pen.tile([P, SH], F16)
            nc.vector._custom_dve(AFFINE_THEN_ADD, out=v_t,
                                  in0=u_t[:, 2:SH + 2], in1=u_t[:, 0:SH],
                                  s0=16.0, s1=0.0)
            # pair sums: is_equal masks with add-accumulate
            # pair2: u_j == 9  <=> (s_j, s_{j+1}) == (1, 2)
            nc.vector.tensor_scalar(out=junk[:, 0:SH], in0=u_t[:, 0:SH],
                                    scalar1=9.0, scalar2=None, op0=OP.is_equal,
                                    op1=OP.add, accum_out=acc_t[:, 1:2])
            # pair3: u_j + 16 s_{j+2} == 45  <=> (1, 3, 2)
            w6_t = pen.tile([P, SH], F16)
            nc.vector._custom_dve(AFFINE_THEN_ADD, out=w6_t,
                                  in0=s_sb[:, 2:SH + 2], in1=u_t[:, 0:SH],
                                  s0=16.0, s1=0.0)
            nc.vector.tensor_scalar(out=junk[:, 0:SH], in0=w6_t, scalar1=45.0,
                                    scalar2=None, op0=OP.is_equal,
                                    op1=OP.add, accum_out=acc_t[:, 2:3])
            # pair4: v_j == 189  <=> (1, 3, 3, 2)
            nc.vector.tensor_scalar(out=junk[:, 0:SH], in0=v_t, scalar1=189.0,
                                    scalar2=None, op0=OP.is_equal,
                                    op1=OP.add, accum_out=acc_t[:, 3:4])
            nc.vector.memset(acc_t[:, 6:8], 0.0)
            nc.sync.dma_start(out=acc_d, in_=acc_t)

            # -------- dump psums --------
            wps_sb = accp.tile([64, 512], F32)
            nc.scalar.activation(wps_sb, w_ps, AF.Copy)
            nc.sync.dma_start(out=wps_d, in_=wps_sb)
            sps_sb = accp.tile([P, P], F32)
            nc.scalar.activation(sps_sb, s_ps, AF.Copy)
            nc.sync.dma_start(out=sps_d, in_=sps_sb)

    if compile:
        nc.compile()
    return nc


_program = None


def _get_program():
    global _program
    if _program is None:
        _program = build_program()
    return _program


def _pair_boundary(s):
    """The only clamped boundary pair term not covered on device:
    4 * [s[S-3]==1][s[S-2]==3][s[S-1]==2] per row."""
    m = (s[:, -3] == 1) & (s[:, -2] == 3) & (s[:, -1] == 2)
    return 4.0 * float(m.sum())


def combine_partials(results, s_full, ce_weights):
    """Host-side (float64) combination of per-core device partials."""
    w = np.asarray(ce_weights, np.float64)
    Wc = np.zeros(C, np.float64)
    Sc = np.zeros(C, np.float64)
    z0 = 0.0
    pen = 0.0
    j64 = np.arange(64)
    j16 = np.arange(16)
    for r in results:
        wps = r["w_ps"].astype(np.float64)      # [64, 512] = [j, c*64+j]
        for c in range(C):
            Wc[c] += wps[j64, c * 64 + j64].sum()
        sps = r["s_ps"].astype(np.float64)      # [128, 128]
        for c in range(C):
            # psum[jj*8+c, c*16+jj] over jj in [0,16)
            Sc[c] += sps[j16 * 8 + c, c * 16 + j16].sum()
        a = r["acc"].astype(np.float64)         # [128, 8]
        z0 += a[:, 0].sum()
        pen += 2.0 * a[:, 1].sum() + 3.0 * a[:, 2].sum() + 4.0 * a[:, 3].sum()
        mpa = np.minimum(0.0, a[0:RB, 4])
        mpb = np.minimum(0.0, a[RB:P, 4])
        pfa, pfb = a[0:RB, 5], a[RB:P, 5]
        pen += (pfa + pfb - 2.0 * np.minimum(mpa, pfa + mpb)).sum()
    pen += _pair_boundary(s_full)
    ce_loss = float((w * (Wc - Sc)).sum()) / (B * S)
    nnz = B * S - z0
    penalty = pen / nnz
    return np.float32(ce_loss + PENALTY_WEIGHT * penalty)


def make_in_maps(logits, targets, predicted_structures):
    lg = np.ascontiguousarray(logits, dtype=np.float32)
    t = np.ascontiguousarray(targets, dtype=np.uint16)
    s = np.ascontiguousarray(predicted_structures.reshape(B, S), dtype=np.uint16)
    # penalty layout: partition r = first half (real halo), 64+r = second
    # half (zero halo, clamp handled on host)
    sp = np.zeros((NCORES, P, SW), np.uint16)
    in_maps = []
    for core in range(NCORES):
        rows = slice(core * RB, (core + 1) * RB)
        sc = s[rows]
        sp[core, 0:RB, :] = sc[:, 0:SW]
        sp[core, RB:P, 0:SH] = sc[:, SH:S]
        in_maps.append({
            "logits": lg[rows].reshape(P, NP * C),
            "targets": t[rows].reshape(P, NP),
            "structs": sp[core],
        })
    return in_maps, s


def kernel(logits, targets, predicted_structures, ce_weights):
    in_maps, s = make_in_maps(logits, targets, predicted_structures)
    nc = _get_program()
    res = run_bass_kernel_spmd(nc, in_maps, core_ids=list(range(NCORES)))
    return combine_partials(res.results, s, ce_weights)
